# revision 1
# baseline (speedup 1.0000x reference)
"""Multi-head attention (12 heads, RoPE, causal SDPA) for Trainium2, 8 cores.

Sharding: batch (2) x head-group (4 groups of 3 heads). Each core computes,
for its (batch b, head-group hg): QKV projection for its 3 heads, RoPE,
causal attention, and a partial out-projection [T, C] restricted to its
heads' rows of w_out. The host sums the 4 head-group partials per batch.

All matmuls and SBUF-resident tensors are bf16 (PSUM accumulation stays
f32), which halves HBM traffic and SBUF footprint vs f32 and avoids the
fp32r small-tile penalty. Device-side layouts (T=2048, C=768, D=64/head):

  xT    [128, 6, 2048]  x[b].T by contraction chunk (c on partitions)
  wqk   [128, 6, 384]   lhsT weights per chunk: cols [q0|q1][k0|k1][q2|k2]
  wv    [128, 6, 192]   V weights as matmul rhs (3 heads)
  cosT  [128, 2048]     RoPE cos, stacked twice (64 d x 2)
  sinT  [128, 2048]     RoPE sin, stacked twice
  rT    [128, 128]      rotate_half as matmul lhsT (runs on PE)
  tri   [128, 128]      tri[kr, qc] = 1 if qc >= kr (causal keep-mask)

Attention: scores are computed transposed (S^T[k, q] = K Q^T) so softmax
exp lands in [k, q] layout with no max-subtraction (scores are O(1) by
construction). P@V runs in natural layout with P^T as the stationary
operand: out[q, 65] blocks at 65 cycles per 128x128 tile, where column 64
(an all-ones column appended to V) accumulates the softmax denominator for
free. Normalization is then a native per-partition divide. The normalized
attention output [q, d] is transposed back to [d, t] for the out-projection
with the DMA engines' XBAR transpose (14 ns/tile, off the compute engines).

Emission is software-pipelined for the in-order engines: scores for heads
0/1 of q-group 0 are emitted right after their weight chunks so the ACT
engine (exp is the second-busiest stream) starts early; the group-1 score
loops are interleaved with P@V and out-projection units so the PE has work
while exp catches up.
"""
import numpy as np

B, T, C, H, D = 2, 2048, 768, 12, 64
HPG = 3                    # heads per group
NG = B * (H // HPG)        # 8 cores
ROPE_BASE = 10000.0
TQ = T // 128              # 16 t-tiles
NCC = C // 128             # 6 contraction chunks
GW = 1024                  # attention q-group width
NGRP = T // GW             # 2 q-groups

_CACHE = {}


def _build_nc(reps=1):
    from concourse import bacc, tile, mybir

    f32 = mybir.dt.float32
    bf16 = mybir.dt.bfloat16
    Exp = mybir.ActivationFunctionType.Exp
    mult = mybir.AluOpType.mult
    add = mybir.AluOpType.add
    div = mybir.AluOpType.divide

    nc = bacc.Bacc("TRN2", target_bir_lowering=False, debug=False,
                   num_devices=NG)

    xT_d = nc.dram_tensor("xT", [C, T], bf16, kind="ExternalInput").ap()
    wqk_d = nc.dram_tensor("wqk", [C, 384], bf16, kind="ExternalInput").ap()
    wv_d = nc.dram_tensor("wv", [C, 192], bf16, kind="ExternalInput").ap()
    woA_d = nc.dram_tensor("woA", [2 * D, C], bf16, kind="ExternalInput").ap()
    woB_d = nc.dram_tensor("woB", [D, C], bf16, kind="ExternalInput").ap()
    cosT_d = nc.dram_tensor("cosT", [128, T], bf16, kind="ExternalInput").ap()
    sinT_d = nc.dram_tensor("sinT", [128, T], bf16, kind="ExternalInput").ap()
    rT_d = nc.dram_tensor("rT", [128, 128], bf16, kind="ExternalInput").ap()
    tri_d = nc.dram_tensor("tri", [128, 128], bf16, kind="ExternalInput").ap()
    out_d = nc.dram_tensor("out", [T, C], bf16, kind="ExternalOutput").ap()

    with tile.TileContext(nc) as tc:
      for rep in range(reps):
        with tc.tile_pool(name=f"persist{rep}", bufs=1) as pp:
            # ---- persistent tiles + constant loads ----
            # big loads on SP; small early ones on ACT (idle until exp)
            wqk = pp.tile([128, NCC, 384], bf16, tag="wqk")
            nc.sync.dma_start(
                wqk[:], wqk_d.rearrange("(c p) i -> p c i", p=128))
            xT = pp.tile([128, NCC, T], bf16, tag="xT")
            nc.scalar.dma_start(
                xT[:, 0:3, 0:512],
                xT_d[0:384, 0:512].rearrange("(c p) i -> p c i", p=128))
            nc.sync.dma_start(
                xT[:, 3:6, 0:512],
                xT_d[384:768, 0:512].rearrange("(c p) i -> p c i", p=128))
            for n in range(1, 4):
                nsl = slice(512 * n, 512 * (n + 1))
                nc.sync.dma_start(
                    xT[:, :, nsl],
                    xT_d[:, nsl].rearrange("(c p) i -> p c i", p=128))
            cosT = pp.tile([128, T], bf16, tag="cosT")
            nc.scalar.dma_start(cosT[:], cosT_d[:])
            sinT = pp.tile([128, T], bf16, tag="sinT")
            nc.scalar.dma_start(sinT[:], sinT_d[:])
            rT = pp.tile([128, 128], bf16, tag="rT")
            nc.scalar.dma_start(rT[:], rT_d[:])
            wv = pp.tile([128, NCC, 192], bf16, tag="wv")
            nc.sync.dma_start(wv[:], wv_d.rearrange("(c p) i -> p c i", p=128))
            tri = pp.tile([128, 128], bf16, tag="tri")
            nc.scalar.dma_start(tri[:], tri_d[:])
            woA = pp.tile([2 * D, C], bf16, tag="woA")
            nc.scalar.dma_start(woA[:], woA_d[:])
            woB = pp.tile([D, C], bf16, tag="woB")
            nc.scalar.dma_start(woB[:], woB_d[:])

            qk_rows = [128, 128, 64, 64]
            qkT = [pp.tile([qk_rows[m], T], bf16, tag=f"qkT{m}",
                           name=f"qkT{m}") for m in range(4)]
            v_sb = pp.tile([128, TQ, HPG, 65], bf16, tag="v_sb")
            nc.gpsimd.memset(v_sb[:, :, :, 64:65], 1.0)
            attn_sb = pp.tile([128, TQ, 256], bf16, tag="attn_sb")
            nc.gpsimd.memset(attn_sb[:, :, 192:256], 0.0)
            attnT = pp.tile([128, TQ * 2, 128], bf16, tag="attnT")

            wp = tc.tile_pool(name=f"work{rep}", bufs=1)
            wk = wp.__enter__()
            ps_pool = tc.tile_pool(name=f"ps{rep}", bufs=4, space="PSUM")
            psp = ps_pool.__enter__()     # <=512 f32: praw/prot/pv/pos/pout
            pscr_pool = tc.tile_pool(name=f"pscr{rep}", bufs=2, space="PSUM")
            pscrp = pscr_pool.__enter__()          # [128, 1024] score tiles

            # ---- QKV projection + RoPE (rotate-half on PE) ----
            raws = {}

            def emit_raw(m, n):
                nsl = slice(512 * n, 512 * (n + 1))
                praw = psp.tile([128, 512], f32, tag="ps", name=f"praw{m}_{n}")
                for c in range(NCC):
                    nc.tensor.matmul(
                        praw[:], wqk[:, c, 128 * m:128 * (m + 1)],
                        xT[:, c, nsl], start=(c == 0), stop=(c == NCC - 1))
                raw = wk.tile([128, 512], bf16, tag="raw", bufs=4,
                              name=f"raw{m}_{n}")
                if m < 2 and n == 1:
                    nc.scalar.copy(raw[:], praw[:])
                else:
                    nc.vector.tensor_copy(raw[:], praw[:])
                raws[(m, n)] = raw

            def emit_rope(m, n):
                nsl = slice(512 * n, 512 * (n + 1))
                raw = raws.pop((m, n))
                prot = psp.tile([128, 512], f32, tag="ps", name=f"prot{m}_{n}")
                nc.tensor.matmul(prot[:], rT[:], raw[:], start=True, stop=True)
                t2 = wk.tile([128, 512], bf16, tag="t2", bufs=3,
                             name=f"t2_{m}_{n}")
                nc.vector.tensor_tensor(t2[:], prot[:], sinT[:, nsl], mult)
                t1 = wk.tile([128, 512], bf16, tag="t1", bufs=3,
                             name=f"t1_{m}_{n}")
                nc.vector.tensor_tensor(t1[:], raw[:], cosT[:, nsl], mult)
                if m < 2:
                    nc.vector.tensor_tensor(qkT[m][:, nsl], t1[:], t2[:], add)
                else:
                    nc.vector.tensor_tensor(qkT[2][:, nsl], t1[0:64, :],
                                            t2[0:64, :], add)
                    nc.vector.tensor_tensor(qkT[3][:, nsl], t1[64:128, :],
                                            t2[64:128, :], add)

            def emit_qk(ms, ns):
                chunks = [(m, n) for n in ns for m in ms]
                for i, (m, n) in enumerate(chunks):
                    emit_raw(m, n)
                    if i >= 1:
                        emit_rope(*chunks[i - 1])
                emit_rope(*chunks[-1])

            def qk_units(ms, ns):
                """emit_qk split into filler-sized closures (one per chunk)."""
                chunks = [(m, n) for n in ns for m in ms]

                def unit(i):
                    m, n = chunks[i]
                    emit_raw(m, n)
                    if i >= 1:
                        emit_rope(*chunks[i - 1])
                    if i == len(chunks) - 1:
                        emit_rope(m, n)
                return [lambda i=i: unit(i) for i in range(len(chunks))]

            def emit_v(ts):
                for t in ts:
                    tsl = slice(128 * t, 128 * (t + 1))
                    pv = psp.tile([128, 192], f32, tag="ps", name=f"pv{t}")
                    for c in range(NCC):
                        nc.tensor.matmul(pv[:], xT[:, c, tsl], wv[:, c, :],
                                         start=(c == 0), stop=(c == NCC - 1))
                    nc.vector.tensor_copy(
                        v_sb[:, t, :, 0:64],
                        pv[:].rearrange("p (h d) -> p h d", d=64))

            # q/k row views per head: (tile index, partition offset)
            qv = [(0, 0), (0, 64), (2, 0)]
            kv = [(1, 0), (1, 64), (3, 0)]

            def emit_a_unit(g, h, j, pt):
                """Scores + exp + causal mask for one (head, k-block)."""
                qm, qo = qv[h]
                km, ko = kv[h]
                qT = qkT[qm][qo:qo + 64, :]
                kT = qkT[km][ko:ko + 64, :]
                dj = j - (GW // 128) * g
                col0 = 128 * dj if dj >= 0 else 0
                pscr = pscrp.tile([128, GW], f32, tag="pscr",
                                  name=f"pscr{g}_{h}_{j}")
                for s0 in range(col0 - col0 % 512, GW, 512):
                    a0 = max(s0, col0)
                    nc.tensor.matmul(
                        pscr[:, a0:s0 + 512],
                        kT[:, 128 * j:128 * (j + 1)],
                        qT[:, GW * g + a0:GW * g + s0 + 512],
                        start=True, stop=True)
                nc.scalar.activation(pt[:, j, col0:], pscr[:, col0:],
                                     Exp, scale=0.125)
                if dj >= 0:
                    nc.gpsimd.tensor_tensor(
                        pt[:, j, col0:col0 + 128],
                        pt[:, j, col0:col0 + 128], tri[:], mult)

            def alloc_pt(g):
                nj = (GW // 128) * (g + 1)
                return wk.tile([128, nj, GW], bf16, tag=f"ptg{g}", bufs=2,
                               name=f"pt{g}")

            def emit_b_unit(g, h, qcl, pt):
                """P^T-stationary PV + denominator + normalize (one q-chunk)."""
                qq = (GW // 128) * g + qcl      # global q-block
                pos = psp.tile([128, 65], f32, tag="ps",
                               name=f"pos{g}_{h}_{qcl}")
                for j in range(qq + 1):
                    nc.tensor.matmul(
                        pos[:], pt[:, j, 128 * qcl:128 * (qcl + 1)],
                        v_sb[:, j, h, :], start=(j == 0), stop=(j == qq))
                rden = wk.tile([128, 1], f32, tag="rden", bufs=4,
                               name=f"rden{g}_{h}_{qcl}")
                nc.vector.reciprocal(rden[:], pos[:, 64:65])
                nc.vector.tensor_scalar(
                    attn_sb[:, qq, 64 * h:64 * (h + 1)], pos[:, 0:64],
                    rden[:], None, mult)

            def emit_transposes(tq0, ntq):
                nc.sync.dma_start_transpose(
                    attnT[:, 2 * tq0:2 * (tq0 + ntq), :],
                    attn_sb[:, tq0:tq0 + ntq, :])

            osb_t = {}

            def emit_store(tq):
                nc.sync.dma_start(out_d[128 * tq:128 * (tq + 1), :],
                                  osb_t.pop(tq)[:])

            def emit_outproj(tq, store=True):
                osb = wk.tile([128, C], bf16, tag="osb", bufs=4,
                              name=f"osb{tq}")
                osb_t[tq] = osb
                for c0, cn in ((0, 512), (512, 256)):
                    pout = psp.tile([128, cn], f32, tag="ps",
                                    name=f"pout{tq}_{c0}")
                    nc.tensor.matmul(pout[:], attnT[:, 2 * tq, :],
                                     woA[:, c0:c0 + cn], start=True,
                                     stop=False)
                    nc.tensor.matmul(pout[:], attnT[0:64, 2 * tq + 1, :],
                                     woB[:, c0:c0 + cn], start=False,
                                     stop=True)
                    if tq < 8:
                        nc.vector.tensor_copy(osb[:, c0:c0 + cn], pout[:])
                    else:
                        nc.scalar.copy(osb[:, c0:c0 + cn], pout[:])
                if store:
                    emit_store(tq)

            def emit_a_head(g, h, pt, fillers=()):
                """Emit all scores of (g, h), interleaving filler units."""
                nj = (GW // 128) * (g + 1)
                fillers = list(fillers)
                done = 0
                for j in range(nj):
                    emit_a_unit(g, h, j, pt)
                    want = (j + 1) * len(fillers) // nj
                    while done < want:
                        fillers[done]()
                        done += 1

            # ---- emission schedule ----
            # The three g=1 score loops are the ACT (exp) backbone; all other
            # PE work rides inside them as fillers so neither engine starves.
            emit_qk((0, 1), (0, 1))            # q01/k01 for q-group 0
            u23 = qk_units((0, 1, 2), (2, 3))  # q-group 1 columns
            pt00 = alloc_pt(0)
            emit_a_head(0, 0, pt00,
                        fillers=qk_units((2,), (0, 1)) + u23[:2])
            pt01 = alloc_pt(0)
            emit_a_head(0, 1, pt01, fillers=u23[2:])
            pt10 = alloc_pt(1)
            emit_a_head(1, 0, pt10,
                        fillers=[lambda t=t: emit_v((t,))
                                 for t in range(TQ)])
            pt11 = alloc_pt(1)
            emit_a_head(1, 1, pt11,
                        fillers=[lambda q=q: emit_b_unit(0, 0, q, pt00)
                                 for q in range(8)]
                        + [lambda q=q: emit_b_unit(1, 0, q, pt10)
                           for q in range(8)])
            pt02 = alloc_pt(0)
            emit_a_head(0, 2, pt02,
                        fillers=[lambda q=q: emit_b_unit(0, 1, q, pt01)
                                 for q in range(8)])

            def finish_tq(tq, g, qcl, pt):
                emit_b_unit(g, 2, qcl, pt)      # last head for this q-chunk
                emit_transposes(tq, 1)
                emit_outproj(tq)

            fillers12 = []
            for q in range(8):
                fillers12.append(lambda q=q: emit_b_unit(1, 1, q, pt11))
                fillers12.append(lambda q=q: finish_tq(q, 0, q, pt02))
            pt12 = alloc_pt(1)
            emit_a_head(1, 2, pt12, fillers=fillers12)
            # tail: per q-chunk, PV of last head -> transpose -> out-proj,
            # software-pipelined (lag 2) so PE covers the cross-engine
            # norm->transpose latency with the next chunks' PV work
            for qcl in range(8):
                emit_b_unit(1, 2, qcl, pt12)
                if qcl >= 2:
                    emit_transposes(8 + qcl - 2, 1)
                    emit_outproj(8 + qcl - 2, store=False)
                if qcl >= 4:
                    emit_store(8 + qcl - 4)
            for tq in (14, 15):
                emit_transposes(tq, 1)
                emit_outproj(tq, store=False)
            for tq in (12, 13, 14, 15):
                emit_store(tq)

            pscr_pool.__exit__(None, None, None)
            ps_pool.__exit__(None, None, None)
            wp.__exit__(None, None, None)

    nc.compile()
    return nc


def _host_inputs(x, w_qkv, w_out):
    """Build the 8 per-core input maps (all device tensors bf16)."""
    import ml_dtypes
    bf = ml_dtypes.bfloat16

    inv_freq = 1.0 / (ROPE_BASE ** (np.arange(0, D, 2, dtype=np.float64) / D))
    t = np.arange(T, dtype=np.float64)
    freqs = t[:, None] * inv_freq[None, :]          # [T, D/2]
    emb = np.concatenate([freqs, freqs], axis=-1)   # [T, D]
    cosT = np.ascontiguousarray(np.cos(emb).T.astype(np.float32))  # [D, T]
    sinT = np.ascontiguousarray(np.sin(emb).T.astype(np.float32))
    cosT2 = np.concatenate([cosT, cosT], axis=0).astype(bf)    # [128, T]
    sinT2 = np.concatenate([sinT, sinT], axis=0).astype(bf)

    # rotate_half permutation as matmul lhsT: rot = R @ q, lhsT = R.T
    R = np.zeros((D, D), np.float32)
    R[0:D // 2, D // 2:D] = -np.eye(D // 2)
    R[D // 2:D, 0:D // 2] = np.eye(D // 2)
    R2 = np.zeros((128, 128), np.float32)
    R2[0:64, 0:64] = R
    R2[64:128, 64:128] = R
    rT = np.ascontiguousarray(R2.T).astype(bf)

    tri = np.zeros((128, 128), np.float32)
    for kr in range(128):
        tri[kr, kr:] = 1.0
    tri = tri.astype(bf)

    wq = w_qkv[0:C]
    wk = w_qkv[C:2 * C]
    wv = w_qkv[2 * C:3 * C]

    maps = []
    for core in range(NG):
        b, hg = core // 4, core % 4
        hs = slice(HPG * D * hg, HPG * D * (hg + 1))   # 192 rows of this group
        h2 = HPG * D * hg + 2 * D
        q01 = wq[hs][0:128]                             # [128, C]
        k01 = wk[hs][0:128]
        q2 = wq[h2:h2 + D]
        k2 = wk[h2:h2 + D]
        v3 = wv[hs]                                     # [192, C]
        wqk_a = np.zeros((C, 384), np.float32)
        wqk_a[:, 0:128] = q01.T
        wqk_a[:, 128:256] = k01.T
        wqk_a[:, 256:320] = q2.T
        wqk_a[:, 320:384] = k2.T
        wv_a = np.ascontiguousarray(v3.T)               # [C, 192]
        wo_h = [w_out[:, HPG * D * hg + D * h: HPG * D * hg + D * (h + 1)].T
                for h in range(HPG)]                    # 3 x [64, C]
        woA = np.concatenate([wo_h[0], wo_h[1]], axis=0)  # [128, C]
        woB = wo_h[2]                                     # [64, C]
        maps.append({
            "xT": np.ascontiguousarray(x[b].T).astype(bf),
            "wqk": wqk_a.astype(bf),
            "wv": wv_a.astype(bf),
            "woA": np.ascontiguousarray(woA).astype(bf),
            "woB": np.ascontiguousarray(woB).astype(bf),
            "cosT": cosT2, "sinT": sinT2,
            "rT": rT, "tri": tri,
        })
    return maps


def kernel(x, w_qkv, w_out):
    from concourse.bass_utils import run_bass_kernel_spmd

    if "nc" not in _CACHE:
        _CACHE["nc"] = _build_nc()
    nc = _CACHE["nc"]

    maps = _host_inputs(np.asarray(x, np.float32),
                        np.asarray(w_qkv, np.float32),
                        np.asarray(w_out, np.float32))
    res = run_bass_kernel_spmd(nc, maps, core_ids=list(range(NG))).results
    parts = np.stack([np.asarray(r["out"], dtype=np.float32)
                      for r in res])                    # [8, T, C]
    out = np.zeros((B, T, C), np.float32)
    for b in range(B):
        out[b] = parts[4 * b:4 * (b + 1)].sum(axis=0)
    return out



# revision 22
# speedup vs baseline: 1.3204x; 1.3204x over previous
"""Multi-head attention (12 heads, RoPE, causal SDPA) for Trainium2, 8 cores.

Sharding: batch (2) x head-group (4 groups of 3 heads). Each core computes,
for its (batch b, head-group hg): QKV projection for its 3 heads, RoPE,
causal attention, and a partial out-projection [T, C] restricted to its
heads' rows of w_out. The host sums the 4 head-group partials per batch.

Matmuls and SBUF tensors are bf16 (PSUM f32) except the score matmuls,
which run in fp8e4 with DoubleRow perf mode: Q/K are stored as 8*rope(q)
in fp8 (quant error lands only on the softmax logits, where it is damped
to ~1e-2 final relative error), and each score matmul pairs the real
64-row contraction with a zeroed second k-tile, halving PE cost per
output column. Device-side layouts (T=2048, C=768, D=64/head):

  xT    [128, 6, 2048]  x[b].T by contraction chunk (c on partitions)
  wqk   [128, 6, 384]   lhsT weights per chunk: cols [q0|q1][k0|k1][q2|k2]
  qq    [128, 2560]     fp8 8*rope(q) heads 0/1 + 512-col zero tail
  kk    [128, 2176]     fp8 8*rope(k) heads 0/1 + 128-col zero tail
  qq2/kk2 [64, ...]     same for head 2
  cosT  [128, 2048]     8*cos, stacked twice (64 d x 2); sinT likewise
  rT    [128, 128]      rotate_half as matmul lhsT (runs on PE)
  tri   [128, 128]      tri[kr, qc] = 1 if qc >= kr (causal keep-mask)

Attention: scores are computed transposed (S^T[k, q] = K Q^T) so softmax
exp lands in [k, q] layout with no max-subtraction. P@V runs with P^T as
the stationary operand: out[q, 65] blocks where column 64 (an all-ones
column appended to V) accumulates the softmax denominator for free.
Normalization is a single broadcast-divide per (head, q-block). The
attention output is transposed back to [d, t] with the DMA XBAR.

Engine assignment: PE all matmuls; ACT only the exp stream (the critical
resource at ~60us); DVE the PSUM drains (raw/v/t2/osb) and divides; Pool
(gpsimd) the SBUF-only work (t1 = raw*cos, rope adds, tri masks, memsets);
SP all DMA issues. Emission weaves PE/DVE/Pool units between score tiles
so the ACT exp stream never starves, and the last head of the second
q-group streams PV -> divide -> transpose -> out-projection -> store
inside its own score loop so the kernel tail is only ~2 q-blocks long.
"""
import numpy as np

B, T, C, H, D = 2, 2048, 768, 12, 64
HPG = 3                    # heads per group
NG = B * (H // HPG)        # 8 cores
ROPE_BASE = 10000.0
TQ = T // 128              # 16 t-tiles
NCC = C // 128             # 6 contraction chunks
GW = 1024                  # attention q-group width
QSC = 8.0                  # q/k fp8 pre-scale; exp scale = 1/(8*QSC*QSC)

_CACHE = {}


def _build_nc(reps=1):
    from concourse import bacc, tile, mybir
    from concourse.ap import AP

    f32 = mybir.dt.float32
    bf16 = mybir.dt.bfloat16
    fp8 = mybir.dt.float8e4
    Exp = mybir.ActivationFunctionType.Exp
    mult = mybir.AluOpType.mult
    add = mybir.AluOpType.add
    div = mybir.AluOpType.divide
    DR = mybir.MatmulPerfMode.DoubleRow

    nc = bacc.Bacc("TRN2", target_bir_lowering=False, debug=False,
                   num_devices=NG)

    xT_d = nc.dram_tensor("xT", [C, T], bf16, kind="ExternalInput").ap()
    wqk_d = nc.dram_tensor("wqk", [C, 384], bf16, kind="ExternalInput").ap()
    wv_d = nc.dram_tensor("wv", [C, 192], bf16, kind="ExternalInput").ap()
    woA_d = nc.dram_tensor("woA", [2 * D, C], bf16, kind="ExternalInput").ap()
    woB_d = nc.dram_tensor("woB", [D, C], bf16, kind="ExternalInput").ap()
    cosT_d = nc.dram_tensor("cosT", [128, T], bf16, kind="ExternalInput").ap()
    sinT_d = nc.dram_tensor("sinT", [128, T], bf16, kind="ExternalInput").ap()
    rT_d = nc.dram_tensor("rT", [128, 128], bf16, kind="ExternalInput").ap()
    tri_d = nc.dram_tensor("tri", [128, 128], bf16, kind="ExternalInput").ap()
    out_d = nc.dram_tensor("out", [T, C], bf16, kind="ExternalOutput").ap()

    escale = 0.125 / (QSC * QSC)

    with tile.TileContext(nc) as tc:
      for rep in range(reps):
        with tc.tile_pool(name=f"persist{rep}", bufs=1) as pp:
            # ---- persistent tiles + constant loads ----
            # three DMA queues run in parallel (transfers serialize per
            # issuing queue); ACT issues only the early-critical loads so
            # it frees up for the prologue drains + first exps
            wqk = pp.tile([128, NCC, 384], bf16, tag="wqk")
            nc.scalar.dma_start(
                wqk[:, :, 0:256],
                wqk_d[:, 0:256].rearrange("(c p) i -> p c i", p=128))
            xT = pp.tile([128, NCC, T], bf16, tag="xT")
            nc.sync.dma_start(
                xT[:, 0:3, 0:512],
                xT_d[0:384, 0:512].rearrange("(c p) i -> p c i", p=128))
            sinT = pp.tile([128, T], bf16, tag="sinT")
            nc.gpsimd.dma_start(sinT[:, 0:1024], sinT_d[:, 0:1024])
            cosT = pp.tile([128, T], bf16, tag="cosT")
            nc.scalar.dma_start(cosT[:, 0:1024], cosT_d[:, 0:1024])
            rT = pp.tile([128, 128], bf16, tag="rT")
            nc.scalar.dma_start(rT[:], rT_d[:])
            tri = pp.tile([128, 128], bf16, tag="tri")
            nc.scalar.dma_start(tri[:], tri_d[:])
            nc.sync.dma_start(
                xT[:, 3:6, 0:512],
                xT_d[384:768, 0:512].rearrange("(c p) i -> p c i", p=128))
            for n in range(1, 4):
                nsl = slice(512 * n, 512 * (n + 1))
                nc.sync.dma_start(
                    xT[:, :, nsl],
                    xT_d[:, nsl].rearrange("(c p) i -> p c i", p=128))

            # fp8 q/k, a zero strip ahead of the data serving as the
            # DoubleRow second tile (in front so score reads never span
            # past their own columns -> no false deps on later writes)
            qq = pp.tile([128, 512 + T], fp8, tag="qq")
            kk = pp.tile([128, 128 + T], fp8, tag="kk")
            qq2 = pp.tile([64, 512 + T], fp8, tag="qq2")
            kk2 = pp.tile([64, 128 + T], fp8, tag="kk2")
            nc.gpsimd.memset(qq[:, 0:512], 0.0)
            nc.gpsimd.memset(kk[:, 0:128], 0.0)
            nc.gpsimd.memset(qq2[:, 0:512], 0.0)
            nc.gpsimd.memset(kk2[:, 0:128], 0.0)

            v_sb = pp.tile([128, TQ, HPG, 65], bf16, tag="v_sb")
            nc.gpsimd.memset(v_sb[:, :, :, 64:65], 1.0)
            attn_sb = pp.tile([128, TQ, 256], bf16, tag="attn_sb")
            nc.gpsimd.memset(attn_sb[:, :, 192:256], 0.0)
            attnT = pp.tile([128, TQ * 2, 128], bf16, tag="attnT")

            # late-needed loads, issued behind the Pool memsets / SP xT
            nc.gpsimd.dma_start(sinT[:, 1024:], sinT_d[:, 1024:])
            nc.gpsimd.dma_start(cosT[:, 1024:], cosT_d[:, 1024:])
            nc.gpsimd.dma_start(
                wqk[:, :, 256:384],
                wqk_d[:, 256:384].rearrange("(c p) i -> p c i", p=128))
            wv = pp.tile([128, NCC, 192], bf16, tag="wv")
            nc.sync.dma_start(wv[:], wv_d.rearrange("(c p) i -> p c i",
                                                    p=128))
            woA = pp.tile([2 * D, C], bf16, tag="woA")
            nc.sync.dma_start(woA[:], woA_d[:])
            woB = pp.tile([D, C], bf16, tag="woB")
            nc.sync.dma_start(woB[:], woB_d[:])

            wp = tc.tile_pool(name=f"work{rep}", bufs=1)
            wk = wp.__enter__()
            ps_pool = tc.tile_pool(name=f"ps{rep}", bufs=4, space="PSUM")
            psp = ps_pool.__enter__()     # <=512 f32: praw/prot/pv/pos/pout
            pscr_pool = tc.tile_pool(name=f"pscr{rep}", bufs=2, space="PSUM")
            pscrp = pscr_pool.__enter__()          # [128, 1024] score tiles

            def dr_ap(t2d, zw, col, width):
                """[P, 2, width] DR pair: tile0 = the zero strip at col 0,
                tile1 = data at col zw+col. Same rows of t2d for both."""
                a = t2d
                return AP(tensor=a.tensor, offset=a.offset,
                          ap=[list(a.ap[0]), [zw + col, 2], [1, width]])

            # q/k row views per head: (tensor, row offset)
            def qk_views(h):
                if h < 2:
                    return (qq[64 * h:64 * (h + 1), :],
                            kk[64 * h:64 * (h + 1), :])
                return qq2[:, :], kk2[:, :]

            # ---- QKV projection + RoPE (rotate-half on PE) ----
            raws = {}
            praws = {}

            def emit_raw_cs(m, n, cs):
                nsl = slice(512 * n, 512 * (n + 1))
                if cs[0] == 0:
                    praws[(m, n)] = psp.tile([128, 512], f32, tag="ps",
                                             name=f"praw{m}_{n}")
                praw = praws[(m, n)]
                for c in cs:
                    nc.tensor.matmul(
                        praw[:], wqk[:, c, 128 * m:128 * (m + 1)],
                        xT[:, c, nsl], start=(c == 0), stop=(c == NCC - 1))
                if cs[-1] == NCC - 1:
                    raw = wk.tile([128, 512], bf16, tag="raw", bufs=4,
                                  name=f"raw{m}_{n}")
                    if (m, n) in ((0, 0), (1, 0), (0, 1), (1, 1)):
                        nc.scalar.copy(raw[:], praws.pop((m, n))[:])
                    else:
                        nc.vector.tensor_copy(raw[:], praws.pop((m, n))[:])
                    raws[(m, n)] = raw

            def emit_raw(m, n):
                emit_raw_cs(m, n, (0, 1, 2, 3, 4, 5))

            def emit_rope(m, n):
                nsl = slice(512 * n, 512 * (n + 1))
                raw = raws.pop((m, n))
                prot = psp.tile([128, 512], f32, tag="ps", name=f"prot{m}_{n}")
                nc.tensor.matmul(prot[:], rT[:], raw[:], start=True, stop=True)
                alt = (m, n) in ((1, 0), (1, 1)) and True
                t1 = wk.tile([128, 512], bf16, tag="t1", bufs=3,
                             name=f"t1_{m}_{n}")
                (nc.vector if alt else nc.gpsimd).tensor_tensor(
                    t1[:], raw[:], cosT[:, nsl], mult)
                t2 = wk.tile([128, 512], bf16, tag="t2", bufs=3,
                             name=f"t2_{m}_{n}")
                nc.vector.tensor_tensor(t2[:], prot[:], sinT[:, nsl], mult)
                qsl = slice(512 + 512 * n, 512 + 512 * (n + 1))
                ksl = slice(128 + 512 * n, 128 + 512 * (n + 1))
                if m == 0:
                    nc.gpsimd.tensor_tensor(qq[:, qsl], t1[:], t2[:], add)
                elif m == 1:
                    eng = nc.vector if n < 2 else nc.gpsimd
                    eng.tensor_tensor(kk[:, ksl], t1[:], t2[:], add)
                else:
                    nc.gpsimd.tensor_tensor(qq2[:, qsl], t1[0:64, :],
                                            t2[0:64, :], add)
                    nc.gpsimd.tensor_tensor(kk2[:, ksl], t1[64:128, :],
                                            t2[64:128, :], add)

            def emit_v(t):
                tsl = slice(128 * t, 128 * (t + 1))
                pv = psp.tile([128, 192], f32, tag="ps", name=f"pv{t}")
                for c in range(NCC):
                    nc.tensor.matmul(pv[:], xT[:, c, tsl], wv[:, c, :],
                                     start=(c == 0), stop=(c == NCC - 1))
                nc.vector.tensor_copy(
                    v_sb[:, t, :, 0:64],
                    pv[:].rearrange("p (h d) -> p h d", d=64))

            def emit_a_unit(g, h, j, pt, split=False):
                """Scores (fp8 DR) + exp + causal mask for one (head,
                k-block). split=True exps each 512-col chunk separately
                (earlier ACT start during the prologue ramp)."""
                qT, kT = qk_views(h)
                dj = j - (GW // 128) * g
                col0 = 128 * dj if dj >= 0 else 0
                pscr = pscrp.tile([128, GW], f32, tag="pscr",
                                  name=f"pscr{g}_{h}_{j}")
                for s0 in range(col0 - col0 % 512, GW, 512):
                    a0 = max(s0, col0)
                    nc.tensor.matmul(
                        pscr[:, a0:s0 + 512],
                        dr_ap(kT, 128, 128 * j, 128),
                        dr_ap(qT, 512, GW * g + a0, s0 + 512 - a0),
                        start=True, stop=True, perf_mode=DR)
                    if split:
                        nc.scalar.activation(pt[:, j, a0:s0 + 512],
                                             pscr[:, a0:s0 + 512],
                                             Exp, scale=escale)
                if not split:
                    nc.scalar.activation(pt[:, j, col0:], pscr[:, col0:],
                                         Exp, scale=escale)
                if dj >= 0:
                    nc.gpsimd.tensor_tensor(
                        pt[:, j, col0:col0 + 128],
                        pt[:, j, col0:col0 + 128], tri[:], mult)

            def alloc_pt(g):
                nj = (GW // 128) * (g + 1)
                return wk.tile([128, nj, GW], bf16, tag=f"ptg{g}", bufs=2,
                               name=f"pt{g}")

            def emit_b_unit(g, h, qcl, pt):
                """P^T-stationary PV + denominator + normalize (one
                q-chunk)."""
                qq_blk = (GW // 128) * g + qcl      # global q-block
                pos = psp.tile([128, 65], f32, tag="ps",
                               name=f"pos{g}_{h}_{qcl}")
                for j in range(qq_blk + 1):
                    nc.tensor.matmul(
                        pos[:], pt[:, j, 128 * qcl:128 * (qcl + 1)],
                        v_sb[:, j, h, :], start=(j == 0), stop=(j == qq_blk))
                nc.vector.tensor_tensor(
                    attn_sb[:, qq_blk, 64 * h:64 * (h + 1)], pos[:, 0:64],
                    pos[:, 64:65].broadcast_to([128, 64]), div)

            def emit_transpose(tq):
                nc.sync.dma_start_transpose(
                    attnT[:, 2 * tq:2 * (tq + 1), :],
                    attn_sb[:, tq:tq + 1, :])

            osb_t = {}

            def emit_store(tq):
                nc.sync.dma_start(out_d[128 * tq:128 * (tq + 1), :],
                                  osb_t.pop(tq)[:])

            def emit_outproj(tq, store=True, drain="dve"):
                osb = wk.tile([128, C], bf16, tag="osb", bufs=8,
                              name=f"osb{tq}")
                osb_t[tq] = osb
                for c0, cn in ((0, 512), (512, 256)):
                    pout = psp.tile([128, cn], f32, tag="ps",
                                    name=f"pout{tq}_{c0}")
                    nc.tensor.matmul(pout[:], attnT[:, 2 * tq, :],
                                     woA[:, c0:c0 + cn], start=True,
                                     stop=False)
                    nc.tensor.matmul(pout[:], attnT[0:64, 2 * tq + 1, :],
                                     woB[:, c0:c0 + cn], start=False,
                                     stop=True)
                    if drain == "dve":
                        nc.vector.tensor_copy(osb[:, c0:c0 + cn], pout[:])
                    else:
                        nc.scalar.copy(osb[:, c0:c0 + cn], pout[:])
                if store:
                    emit_store(tq)

            def finish_tq(tq, g, qcl, pt):
                """PV+transpose for tq, out-projection lagging 2 chains so
                in-order engines keep multiple chains in flight."""
                emit_b_unit(g, 2, qcl, pt)      # last head for this q-chunk
                emit_transpose(tq)
                if tq >= 2:
                    emit_outproj(tq - 2)

            def emit_a_head(g, h, pt, fillers=(), split_first=False):
                """Emit all scores of (g, h), interleaving filler units."""
                nj = (GW // 128) * (g + 1)
                fillers = list(fillers)
                done = 0
                for j in range(nj):
                    emit_a_unit(g, h, j, pt,
                                split=(split_first and j < 2))
                    want = (j + 1) * len(fillers) // nj
                    while done < want:
                        fillers[done]()
                        done += 1

            def qk_unit(m, n):
                """One q/k projection chunk as four fine-grained filler
                closures (~0.2-0.5us of PE each)."""
                return [lambda: emit_raw_cs(m, n, (0, 1)),
                        lambda: emit_raw_cs(m, n, (2, 3)),
                        lambda: emit_raw_cs(m, n, (4, 5)),
                        lambda: emit_rope(m, n)]

            # ---- emission schedule ----
            # Prologue: q/k heads 0/1 for q-group 0 (cols 0..1023).
            emit_raw_cs(0, 0, (0, 1, 2))
            emit_raw_cs(0, 0, (3, 4, 5))
            emit_raw_cs(1, 0, (0, 1, 2))
            emit_raw_cs(1, 0, (3, 4, 5))
            emit_rope(0, 0)
            emit_raw(0, 1)
            emit_rope(1, 0)
            emit_raw(1, 1)
            emit_rope(0, 1)
            emit_rope(1, 1)

            pt00 = alloc_pt(0)
            emit_a_head(0, 0, pt00,
                        fillers=qk_unit(0, 2) + qk_unit(1, 2),
                        split_first=True)
            pt01 = alloc_pt(0)
            emit_a_head(0, 1, pt01,
                        fillers=qk_unit(0, 3) + qk_unit(1, 3))
            pt10 = alloc_pt(1)
            emit_a_head(1, 0, pt10,
                        fillers=qk_unit(2, 0) + qk_unit(2, 1)
                        + qk_unit(2, 2) + qk_unit(2, 3)
                        + [lambda t=t: emit_v(t) for t in range(8)]
                        + [lambda q=q: emit_b_unit(0, 0, q, pt00)
                           for q in range(4)])
            pt11 = alloc_pt(1)
            emit_a_head(1, 1, pt11,
                        fillers=[lambda q=q: emit_b_unit(0, 0, q, pt00)
                                 for q in (4, 5, 6, 7)]
                        + [lambda t=t: emit_v(t) for t in (8, 9, 10, 11)]
                        + [lambda q=q: emit_b_unit(0, 1, q, pt01)
                           for q in range(8)]
                        + [lambda q=q: emit_b_unit(1, 0, q, pt10)
                           for q in (0, 1)])
            pt02 = alloc_pt(0)
            emit_a_head(0, 2, pt02,
                        fillers=[lambda t=t: emit_v(t)
                                 for t in (12, 13, 14, 15)]
                        + [lambda q=q: emit_b_unit(1, 0, q, pt10)
                           for q in (2, 3, 4, 5, 6, 7)]
                        + [lambda q=q: emit_b_unit(1, 1, q, pt11)
                           for q in (0, 1)])

            # Last head of q-group 1: g0 finishes ride as fillers in the
            # first half; the g1 endgame streams stage-pipelined (PV ->
            # transpose -> outproj -> store, each lagging the exp that
            # unblocks it) so only ~2 chains trail the final exp.
            pt12 = alloc_pt(1)
            f6 = []
            for q in range(8):
                if q < 6:
                    f6.append(lambda q=q: emit_b_unit(1, 1, q + 2, pt11))
                f6.append(lambda q=q: finish_tq(q, 0, q, pt02))
            done6 = 0
            for j in range(16):
                emit_a_unit(1, 2, j, pt12)
                if j < 8:
                    want = (j + 1) * len(f6) // 8
                    while done6 < want:
                        f6[done6]()
                        done6 += 1
                else:
                    emit_b_unit(1, 2, j - 8, pt12)
                    if j == 8:
                        emit_outproj(6)
                    if j == 9:
                        emit_outproj(7)
                    if j >= 10:
                        emit_transpose(j - 2)
                    if j >= 12:
                        emit_outproj(j - 4, store=False,
                                     drain=("act" if j % 2 else "dve"))
                        emit_store(j - 4)
            # Stage-partitioned epilogue: chains for tq 12..15 overlap in
            # flight; drains split ACT/DVE since the exp stream is over.
            for tq in (14, 15):
                emit_transpose(tq)
            for tq in range(12, 16):
                emit_outproj(tq, store=False,
                             drain=("act" if tq % 2 else "dve"))
                emit_store(tq)

            pscr_pool.__exit__(None, None, None)
            ps_pool.__exit__(None, None, None)
            wp.__exit__(None, None, None)

    nc.compile()
    return nc


def _host_inputs(x, w_qkv, w_out):
    """Build the 8 per-core input maps."""
    import ml_dtypes
    bf = ml_dtypes.bfloat16

    inv_freq = 1.0 / (ROPE_BASE ** (np.arange(0, D, 2, dtype=np.float64) / D))
    t = np.arange(T, dtype=np.float64)
    freqs = t[:, None] * inv_freq[None, :]          # [T, D/2]
    emb = np.concatenate([freqs, freqs], axis=-1)   # [T, D]
    cosT = np.ascontiguousarray(np.cos(emb).T.astype(np.float32)) * QSC
    sinT = np.ascontiguousarray(np.sin(emb).T.astype(np.float32)) * QSC
    cosT2 = np.concatenate([cosT, cosT], axis=0).astype(bf)    # [128, T]
    sinT2 = np.concatenate([sinT, sinT], axis=0).astype(bf)

    # rotate_half permutation as matmul lhsT: rot = R @ q, lhsT = R.T
    R = np.zeros((D, D), np.float32)
    R[0:D // 2, D // 2:D] = -np.eye(D // 2)
    R[D // 2:D, 0:D // 2] = np.eye(D // 2)
    R2 = np.zeros((128, 128), np.float32)
    R2[0:64, 0:64] = R
    R2[64:128, 64:128] = R
    rT = np.ascontiguousarray(R2.T).astype(bf)

    tri = np.zeros((128, 128), np.float32)
    for kr in range(128):
        tri[kr, kr:] = 1.0
    tri = tri.astype(bf)

    wq = w_qkv[0:C]
    wk = w_qkv[C:2 * C]
    wv = w_qkv[2 * C:3 * C]

    maps = []
    for core in range(NG):
        b, hg = core // 4, core % 4
        hs = slice(HPG * D * hg, HPG * D * (hg + 1))   # 192 rows of group
        h2 = HPG * D * hg + 2 * D
        q01 = wq[hs][0:128]                             # [128, C]
        k01 = wk[hs][0:128]
        q2 = wq[h2:h2 + D]
        k2 = wk[h2:h2 + D]
        v3 = wv[hs]                                     # [192, C]
        wqk_a = np.zeros((C, 384), np.float32)
        wqk_a[:, 0:128] = q01.T
        wqk_a[:, 128:256] = k01.T
        wqk_a[:, 256:320] = q2.T
        wqk_a[:, 320:384] = k2.T
        wv_a = np.ascontiguousarray(v3.T)               # [C, 192]
        wo_h = [w_out[:, HPG * D * hg + D * h: HPG * D * hg + D * (h + 1)].T
                for h in range(HPG)]                    # 3 x [64, C]
        woA = np.concatenate([wo_h[0], wo_h[1]], axis=0)  # [128, C]
        woB = wo_h[2]                                     # [64, C]
        maps.append({
            "xT": np.ascontiguousarray(x[b].T).astype(bf),
            "wqk": wqk_a.astype(bf),
            "wv": wv_a.astype(bf),
            "woA": np.ascontiguousarray(woA).astype(bf),
            "woB": np.ascontiguousarray(woB).astype(bf),
            "cosT": cosT2, "sinT": sinT2,
            "rT": rT, "tri": tri,
        })
    return maps


def kernel(x, w_qkv, w_out):
    from concourse.bass_utils import run_bass_kernel_spmd

    if "nc" not in _CACHE:
        _CACHE["nc"] = _build_nc()
    nc = _CACHE["nc"]

    maps = _host_inputs(np.asarray(x, np.float32),
                        np.asarray(w_qkv, np.float32),
                        np.asarray(w_out, np.float32))
    res = run_bass_kernel_spmd(nc, maps, core_ids=list(range(NG))).results
    parts = np.stack([np.asarray(r["out"], dtype=np.float32)
                      for r in res])                    # [8, T, C]
    out = np.zeros((B, T, C), np.float32)
    for b in range(B):
        out[b] = parts[4 * b:4 * (b + 1)].sum(axis=0)
    return out
